# revision 1
# baseline (speedup 1.0000x reference)
"""BiaffineLabelAttention kernel for 8 TRN2 NeuronCores (Bass/Tile).

Reference computation (per full input):
    t1[b,l,i,o] = sum_d head[b,i,d] * U[l,d] * dep[b,o,d]
    t2_h[b,l,i] = sum_d W_h[l,d] * head[b,i,d]
    t2_d[b,l,o] = sum_d W_d[l,d] * dep[b,o,d]
    out = t1 + t2_h[...,None] + t2_d[...,None,:] + bias[l]

Sharding: data-parallel over batch (16 batches -> 2 per core x 8 cores).

Per-core algorithm:
    headT[d,i], depT[d,o] built via PE transposes.
    scaled[d,(l,o)] = U[l,d]*depT[d,o] + W_h[l,d]      (DVE/ACT per-partition fma)
    psum[i,(l,o)]  += headT[d,i].T @ scaled[d,(l,o)]   (fp32r matmuls, N=512 = 2 labels)
    out_sb = psum + t2row[(l,o)]                        (DVE tensor_tensor add)
The W_h term telescopes: sum_d headT[d,i]*W_h[l,d] = t2_h[b,l,i].
t2row = t2_d + bias comes from a small PE matmul, flattened to one partition
via a DRAM bounce, then replicated across partitions by per-pair GPSIMD
partition_broadcast chunks (keeps the DMA engines free for real traffic).
"""

import numpy as np
from contextlib import ExitStack

import concourse.bass as bass
from concourse import bacc, mybir, tile, masks
from concourse.bass_utils import run_bass_kernel_spmd

F32 = mybir.dt.float32
F32R = mybir.dt.float32r
BF16 = mybir.dt.bfloat16

B, S, D, L = 16, 256, 768, 32
NCORES = 8
BC = B // NCORES          # batches per core
KT = D // 128             # contraction k-tiles
PAIRS = L // 2            # label pairs sharing one PSUM bank (N=512)
ROWLEN = L * S            # per-batch t2 row length (l,o) flattened

_NC_CACHE = {}


def _build_nc():
    nc = bacc.Bacc(
        "TRN2",
        target_bir_lowering=False,
        debug=False,
        enable_asserts=False,
        num_devices=NCORES,
    )
    head_d = nc.dram_tensor("head", [BC, S, D], F32, kind="ExternalInput")
    dep_d = nc.dram_tensor("dep", [BC, S, D], F32, kind="ExternalInput")
    u_d = nc.dram_tensor("u", [L, D], F32, kind="ExternalInput")
    w_d = nc.dram_tensor("w", [L, 2 * D], F32, kind="ExternalInput")
    b_d = nc.dram_tensor("b", [L, 1], F32, kind="ExternalInput")
    out_d = nc.dram_tensor("out", [BC, L, S, S], F32, kind="ExternalOutput")
    t2_scratch = nc.dram_tensor("t2_scratch", [BC, L, S], F32)

    with tile.TileContext(nc) as tc, ExitStack() as ctx:
        const = ctx.enter_context(tc.tile_pool(name="const", bufs=1))
        big = ctx.enter_context(tc.tile_pool(name="big", bufs=1))
        nat = ctx.enter_context(tc.tile_pool(name="nat", bufs=2))
        scaled_pool = ctx.enter_context(tc.tile_pool(name="scaled", bufs=18))
        outp = ctx.enter_context(tc.tile_pool(name="outp", bufs=6))
        tp_psum = ctx.enter_context(
            tc.tile_pool(name="tp_psum", bufs=2, space=bass.MemorySpace.PSUM)
        )
        t2_psum = ctx.enter_context(
            tc.tile_pool(name="t2_psum", bufs=1, space=bass.MemorySpace.PSUM)
        )
        mm_psum = ctx.enter_context(
            tc.tile_pool(name="mm_psum", bufs=5, space=bass.MemorySpace.PSUM)
        )

        ident = const.tile([128, 128], F32)
        masks.make_identity(nc, ident[:])

        headT = big.tile([128, BC * KT * S], F32R, tag="headT")  # [d, (b,k,i)]
        depT = big.tile([128, BC * KT * S], F32R, tag="depT")    # [d, (b,k,o)]
        ut = big.tile([128, KT * L], F32, tag="ut")    # col k*L+l = U[l, k-blk]
        wht = big.tile([128, KT * L], F32, tag="wht")
        wdt = big.tile([128, KT * L], F32R, tag="wdt")
        bias = const.tile([L, 1], F32, tag="bias")
        # t2bc[p, b*ROWLEN + l*S + o] = t2_d[b,l,o] + bias[l]; the row is
        # DMA'd into partition 0 then partition-broadcast in place
        t2bc = big.tile([128, BC * ROWLEN], F32, tag="t2bc")

        def col(b, k):
            return (b * KT + k) * S

        cp_idx = [0]

        def psum_copy(dst_ap, src_ap):
            # alternate DVE/ACT for PSUM->SBUF evacuation copies
            if cp_idx[0] % 2 == 0:
                nc.vector.tensor_copy(dst_ap, src_ap)
            else:
                nc.scalar.copy(dst_ap, src_ap)
            cp_idx[0] += 1

        nc.sync.dma_start(bias[:], b_d[:])

        def load_transposed(src_d, dst, b):
            # src [S, D] batch b  ->  dst[:, col(b,k) + i] = src[i, k*128+d]
            # 4+2 transposes share one psum bank; one strided copy out
            for ih in range(S // 128):
                natt = nat.tile([128, D], F32, tag="nat")
                half = D // 2
                nc.sync.dma_start(
                    natt[:, :half], src_d[b, ih * 128:(ih + 1) * 128, :half])
                nc.sync.dma_start(
                    natt[:, half:], src_d[b, ih * 128:(ih + 1) * 128, half:])
                for k0, nblk in ((0, 4), (4, 2)):
                    ps = tp_psum.tile([128, 4 * 128], F32, tag="tp")
                    for q in range(nblk):
                        k = k0 + q
                        nc.tensor.transpose(
                            ps[:, q * 128:(q + 1) * 128],
                            natt[:, k * 128:(k + 1) * 128], ident[:],
                        )
                    # dst columns for k0..k0+nblk at this ih: stride S apart
                    out_ap = dst[:].rearrange(
                        "p (bk i) -> p bk i", i=S
                    )[:, b * KT + k0: b * KT + k0 + nblk,
                      ih * 128:(ih + 1) * 128]
                    psum_copy(
                        out_ap,
                        ps[:, :nblk * 128].rearrange(
                            "p (q i) -> p q i", i=128),
                    )

        def load_weightT(src_ap, dst):
            # src [L, D]  ->  dst[:, k*L + l] = src[l, k*128+d]
            natw = nat.tile([L, D], F32, tag="natw")
            nc.sync.dma_start(natw[:], src_ap)
            ps = tp_psum.tile([128, 4 * 128], F32, tag="tp")
            for k in range(KT):
                nc.tensor.transpose(
                    ps[:, k * L:(k + 1) * L],
                    natw[:, k * 128:(k + 1) * 128], ident[:L, :L]
                )
            psum_copy(dst[:], ps[:, :KT * L])

        def t2_chain(b):
            # t2row[b] = t2_d[b] + bias, replicated to all partitions
            ps = t2_psum.tile([L, S], F32, tag="t2")
            for k in range(KT):
                nc.tensor.matmul(
                    ps[:],
                    wdt[:, k * L:(k + 1) * L],
                    depT[:, col(b, k):col(b, k) + S],
                    start=(k == 0),
                    stop=(k == KT - 1),
                )
            t2sb = nat.tile([L, S], F32, tag="t2sb")
            nc.vector.tensor_scalar_add(t2sb[:], ps[:], bias[:])
            nc.sync.dma_start(t2_scratch[b], t2sb[:])
            nc.sync.dma_start(
                t2bc[0:1, b * ROWLEN:(b + 1) * ROWLEN],
                t2_scratch[b].rearrange("l o -> (l o)"),
            )
            for j in range(PAIRS):
                c = b * ROWLEN + j * 2 * S
                nc.gpsimd.partition_broadcast(
                    t2bc[:, c:c + 2 * S], t2bc[0:1, c:c + 2 * S]
                )

        op_idx = [0]

        def scale_op(dst_ap, src_ap, ucol, wcol):
            # scaled = U*depT + W_h; every 8th on GpSimd, rest DVE/ACT ~43/57
            if op_idx[0] % 8 == 7:
                nc.gpsimd.tensor_scalar(
                    dst_ap, src_ap, ucol, wcol,
                    mybir.AluOpType.mult, mybir.AluOpType.add,
                )
            elif (op_idx[0] * 55) % 128 < 55:
                nc.vector.tensor_scalar(
                    dst_ap, src_ap, ucol, wcol,
                    mybir.AluOpType.mult, mybir.AluOpType.add,
                )
            else:
                nc.scalar.activation(
                    dst_ap, src_ap,
                    mybir.ActivationFunctionType.Identity,
                    bias=wcol, scale=ucol,
                )
            op_idx[0] += 1

        def main_pairs(b, jlist=None, split_last=False):
            jl = list(jlist) if jlist is not None else list(range(PAIRS))
            for j in jl:
                split = split_last and j == jl[-1]
                stiles = []
                for k in range(KT):
                    st = scaled_pool.tile([128, 2 * S], F32R, tag="scaled")
                    for h in range(2):
                        lbl = 2 * j + h
                        scale_op(
                            st[:, h * S:(h + 1) * S],
                            depT[:, col(b, k):col(b, k) + S],
                            ut[:, k * L + lbl:k * L + lbl + 1],
                            wht[:, k * L + lbl:k * L + lbl + 1],
                        )
                    stiles.append(st)
                osb = outp.tile([128, 4 * S], F32, tag="osb")  # (l, ib, o)
                osb4 = osb[:].rearrange("i (l ib o) -> i l ib o", l=2, ib=2)
                for ib in range(2):
                    ps = mm_psum.tile([128, 2 * S], F32, tag="mm")
                    for k in range(KT):
                        hc = col(b, k) + ib * 128
                        nc.tensor.matmul(
                            ps[:],
                            headT[:, hc:hc + 128],
                            stiles[k][:],
                            start=(k == 0),
                            stop=(k == KT - 1),
                        )
                    nc.vector.tensor_tensor(
                        osb4[:, :, ib, :],
                        ps[:].rearrange("i (l o) -> i l o", l=2),
                        t2bc[:, b * ROWLEN + j * 2 * S:
                             b * ROWLEN + (j + 1) * 2 * S].rearrange(
                                 "p (l o) -> p l o", l=2),
                        mybir.AluOpType.add,
                    )
                if split:
                    for c in range(4):
                        lh, ib = c // 2, c % 2
                        nc.sync.dma_start(
                            out_d[b, 2 * j + lh,
                                  ib * 128:(ib + 1) * 128, :],
                            osb[:, c * S:(c + 1) * S],
                        )
                else:
                    # one DMA per (b, pair): HBM dim (l,ib) has uniform stride
                    nc.sync.dma_start(
                        out_d[b, 2 * j:2 * j + 2, :, :].rearrange(
                            "l (ib i) o -> i (l ib) o", i=128),
                        osb[:].rearrange("i (lib o) -> i lib o", lib=4),
                    )

        # all transposes + t2 chains first so the 384-matmul main stream
        # then runs uninterrupted (keeps PE dense and HAM warm)
        load_transposed(dep_d, depT, 0)
        load_weightT(w_d[:, D:], wdt)
        t2_chain(0)
        load_weightT(u_d[:], ut)
        load_weightT(w_d[:, :D], wht)
        load_transposed(head_d, headT, 0)
        load_transposed(dep_d, depT, 1)
        t2_chain(1)
        load_transposed(head_d, headT, 1)
        main_pairs(0)
        main_pairs(1, split_last=True)

    nc.compile()
    return nc


def get_nc():
    if "nc" not in _NC_CACHE:
        _NC_CACHE["nc"] = _build_nc()
    return _NC_CACHE["nc"]


def make_in_maps(head, dep, u, w, bvec):
    head = np.ascontiguousarray(np.asarray(head, dtype=np.float32))
    dep = np.ascontiguousarray(np.asarray(dep, dtype=np.float32))
    u = np.ascontiguousarray(np.asarray(u, dtype=np.float32))
    w = np.ascontiguousarray(np.asarray(w, dtype=np.float32))
    bcol = np.ascontiguousarray(
        np.asarray(bvec, dtype=np.float32).reshape(L, 1)
    )
    return [
        {
            "head": head[c * BC:(c + 1) * BC],
            "dep": dep[c * BC:(c + 1) * BC],
            "u": u,
            "w": w,
            "b": bcol,
        }
        for c in range(NCORES)
    ]


def run(head, dep, label_U_diag, label_W, label_b, trace=False, **trace_kw):
    nc = get_nc()
    in_maps = make_in_maps(head, dep, label_U_diag, label_W, label_b)
    res = run_bass_kernel_spmd(
        nc, in_maps, core_ids=list(range(NCORES)), trace=trace, **trace_kw
    )
    out = np.concatenate(
        [res.results[c]["out"] for c in range(NCORES)], axis=0
    )
    return out, res


def kernel(**inputs):
    out, _ = run(
        inputs["head"],
        inputs["dep"],
        inputs["label_U_diag"],
        inputs["label_W"],
        inputs["label_b"],
    )
    return out



# revision 4
# speedup vs baseline: 1.1611x; 1.1611x over previous
"""BiaffineLabelAttention kernel for 8 TRN2 NeuronCores (Bass/Tile).

Reference computation (per full input):
    t1[b,l,i,o] = sum_d head[b,i,d] * U[l,d] * dep[b,o,d]
    t2_h[b,l,i] = sum_d W_h[l,d] * head[b,i,d]
    t2_d[b,l,o] = sum_d W_d[l,d] * dep[b,o,d]
    out = t1 + t2_h[...,None] + t2_d[...,None,:] + bias[l]

Sharding: data-parallel over batch (16 batches -> 2 per core x 8 cores).

Per-core algorithm (bf16 matmul datapath; tolerance is 2e-2 so bf16's
~2e-3 max rel err is fine and the PE runs 1 cycle/row vs fp32's 4):
    headT[d,(b,k,i)], depT[d,(b,k,o)] built via fp32 PE transposes,
        cast to bf16 during the PSUM->SBUF evacuation copy.
    st[j,k][d,(b,l2,o)] = U[l]*depT + W_h[l]   one tensor_scalar per
        (j,k,label) covers BOTH batches (strided out AP), so the MM rhs
        st[:, b*512:(b+1)*512] stays a contiguous (l2,o) block.
    psum[i,(l2,o)] += headT[d,(b,k,i)].T @ st  (bf16 matmuls, N=512)
    osb = psum + t2row  (DVE tensor_tensor, or ACT copy + GpSimd add)
The W_h term telescopes through the matmul; t2row = t2_d + bias comes
from a small PE matmul, flattened to one partition via a DRAM bounce,
then replicated across partitions by GpSimd partition_broadcast (bf16).
Output is written bf16 and upcast to f32 on the host.
"""

import numpy as np
from contextlib import ExitStack

import concourse.bass as bass
from concourse import bacc, mybir, tile, masks
from concourse.bass_utils import run_bass_kernel_spmd

F32 = mybir.dt.float32
BF16 = mybir.dt.bfloat16

B, S, D, L = 16, 256, 768, 32
NCORES = 8
BC = B // NCORES          # batches per core
KT = D // 128             # contraction k-tiles
PAIRS = L // 2            # label pairs sharing one PSUM bank (N=512)
ROWLEN = L * S            # per-batch t2 row length (l,o) flattened

_NC_CACHE = {}


def _build_nc():
    nc = bacc.Bacc(
        "TRN2",
        target_bir_lowering=False,
        debug=False,
        enable_asserts=False,
        num_devices=NCORES,
    )
    head_d = nc.dram_tensor("head", [BC, S, D], F32, kind="ExternalInput")
    dep_d = nc.dram_tensor("dep", [BC, S, D], F32, kind="ExternalInput")
    u_d = nc.dram_tensor("u", [L, D], F32, kind="ExternalInput")
    w_d = nc.dram_tensor("w", [L, 2 * D], F32, kind="ExternalInput")
    b_d = nc.dram_tensor("b", [L, 1], F32, kind="ExternalInput")
    out_d = nc.dram_tensor("out", [BC, L, S, S], BF16, kind="ExternalOutput")
    t2_scratch = nc.dram_tensor("t2_scratch", [BC, L, S], BF16)

    with tile.TileContext(nc) as tc, ExitStack() as ctx:
        const = ctx.enter_context(tc.tile_pool(name="const", bufs=1))
        big = ctx.enter_context(tc.tile_pool(name="big", bufs=1))
        nat = ctx.enter_context(tc.tile_pool(name="nat", bufs=2))
        scaled_pool = ctx.enter_context(tc.tile_pool(name="scaled", bufs=14))
        outp = ctx.enter_context(tc.tile_pool(name="outp", bufs=6))
        evtmp = ctx.enter_context(tc.tile_pool(name="evtmp", bufs=3))
        tp_psum = ctx.enter_context(
            tc.tile_pool(name="tp_psum", bufs=2, space=bass.MemorySpace.PSUM)
        )
        t2_psum = ctx.enter_context(
            tc.tile_pool(name="t2_psum", bufs=1, space=bass.MemorySpace.PSUM)
        )
        mm_psum = ctx.enter_context(
            tc.tile_pool(name="mm_psum", bufs=5, space=bass.MemorySpace.PSUM)
        )

        ident = const.tile([128, 128], F32)
        masks.make_identity(nc, ident[:])

        headT = big.tile([128, BC * KT * S], BF16, tag="headT")  # [d,(b,k,i)]
        depT = big.tile([128, BC * KT * S], BF16, tag="depT")    # [d,(b,k,o)]
        ut = big.tile([128, KT * L], F32, tag="ut")    # col k*L+l = U[l,k-blk]
        wht = big.tile([128, KT * L], F32, tag="wht")
        wdt = big.tile([128, KT * L], BF16, tag="wdt")
        bias = const.tile([L, 1], F32, tag="bias")
        # t2bc[p, b*ROWLEN + l*S + o] = t2_d[b,l,o] + bias[l]; the row is
        # DMA'd into partition 0 then partition-broadcast in place
        t2bc = big.tile([128, BC * ROWLEN], BF16, tag="t2bc")

        def col(b, k):
            return (b * KT + k) * S

        cp_idx = [0]

        def psum_copy(dst_ap, src_ap):
            # alternate DVE/ACT for PSUM->SBUF evacuation copies
            if cp_idx[0] % 2 == 0:
                nc.vector.tensor_copy(dst_ap, src_ap)
            else:
                nc.scalar.copy(dst_ap, src_ap)
            cp_idx[0] += 1

        nc.sync.dma_start(bias[:], b_d[:])

        def load_transposed(src_d, dst, b):
            # src [S, D] batch b  ->  dst[:, col(b,k) + i] = src[i, k*128+d]
            # 4+2 transposes share one psum bank; one strided copy out
            for ih in range(S // 128):
                natt = nat.tile([128, D], F32, tag="nat")
                half = D // 2
                nc.sync.dma_start(
                    natt[:, :half], src_d[b, ih * 128:(ih + 1) * 128, :half])
                nc.sync.dma_start(
                    natt[:, half:], src_d[b, ih * 128:(ih + 1) * 128, half:])
                for k0, nblk in ((0, 4), (4, 2)):
                    ps = tp_psum.tile([128, 4 * 128], F32, tag="tp")
                    for q in range(nblk):
                        k = k0 + q
                        nc.tensor.transpose(
                            ps[:, q * 128:(q + 1) * 128],
                            natt[:, k * 128:(k + 1) * 128], ident[:],
                        )
                    # dst columns for k0..k0+nblk at this ih: stride S apart
                    out_ap = dst[:].rearrange(
                        "p (bk i) -> p bk i", i=S
                    )[:, b * KT + k0: b * KT + k0 + nblk,
                      ih * 128:(ih + 1) * 128]
                    psum_copy(
                        out_ap,
                        ps[:, :nblk * 128].rearrange(
                            "p (q i) -> p q i", i=128),
                    )

        def load_weightT(src_ap, dst):
            # src [L, D]  ->  dst[:, k*L + l] = src[l, k*128+d]
            natw = nat.tile([L, D], F32, tag="natw")
            nc.sync.dma_start(natw[:], src_ap)
            ps = tp_psum.tile([128, 4 * 128], F32, tag="tp")
            for k in range(KT):
                nc.tensor.transpose(
                    ps[:, k * L:(k + 1) * L],
                    natw[:, k * 128:(k + 1) * 128], ident[:L, :L]
                )
            psum_copy(dst[:], ps[:, :KT * L])

        def t2_chain(b):
            # t2row[b] = t2_d[b] + bias, replicated to all partitions
            ps = t2_psum.tile([L, S], F32, tag="t2")
            for k in range(KT):
                nc.tensor.matmul(
                    ps[:],
                    wdt[:, k * L:(k + 1) * L],
                    depT[:, col(b, k):col(b, k) + S],
                    start=(k == 0),
                    stop=(k == KT - 1),
                )
            t2sb = nat.tile([L, S], BF16, tag="t2sb")
            nc.vector.tensor_scalar_add(t2sb[:], ps[:], bias[:])
            nc.sync.dma_start(t2_scratch[b], t2sb[:])
            nc.sync.dma_start(
                t2bc[0:1, b * ROWLEN:(b + 1) * ROWLEN],
                t2_scratch[b].rearrange("l o -> (l o)"),
            )
            for j in range(PAIRS):
                c = b * ROWLEN + j * 2 * S
                nc.gpsimd.partition_broadcast(
                    t2bc[:, c:c + 2 * S], t2bc[0:1, c:c + 2 * S]
                )

        op_idx = [0]

        def scale_op(dst_ap, src_ap, ucol, wcol):
            # st = U*depT + W_h for both batches; DVE/ACT/GpSimd split
            # (DVE gets 4x bf16 packing so it takes the lion's share)
            r = op_idx[0] % 16
            if r == 7:
                nc.gpsimd.tensor_scalar(
                    dst_ap, src_ap, ucol, wcol,
                    mybir.AluOpType.mult, mybir.AluOpType.add,
                )
            elif r in (3, 11, 13, 15):
                nc.scalar.activation(
                    dst_ap, src_ap,
                    mybir.ActivationFunctionType.Identity,
                    bias=wcol, scale=ucol,
                )
            else:
                nc.vector.tensor_scalar(
                    dst_ap, src_ap, ucol, wcol,
                    mybir.AluOpType.mult, mybir.AluOpType.add,
                )
            op_idx[0] += 1

        ev_idx = [0]

        def evac_add(osb_ap, ps, t2_ap):
            # osb = psum + t2row: mostly direct DVE TT (only DVE can
            # read+add from PSUM); every 4th via ACT copy + GpSimd add
            ps_v = ps[:].rearrange("i (l o) -> i l o", l=2)
            if ev_idx[0] % 4 == 3:
                tmp = evtmp.tile([128, 2 * S], BF16, tag="ev")
                nc.scalar.copy(tmp[:], ps[:])
                nc.gpsimd.tensor_tensor(
                    osb_ap,
                    tmp[:].rearrange("i (l o) -> i l o", l=2),
                    t2_ap, mybir.AluOpType.add,
                )
            else:
                nc.vector.tensor_tensor(
                    osb_ap, ps_v, t2_ap, mybir.AluOpType.add,
                )
            ev_idx[0] += 1

        # depT view [p, b, k, o] for the both-batch scale-op source AP
        depT_v = depT[:].rearrange("p (b k o) -> p b k o", b=BC, o=S)

        def main_pair(j, split_last=False):
            # st[k][p, (b, l2, o)]: one tensor_scalar per (k, label)
            # writes both batches' (o) blocks; MM rhs per batch is the
            # contiguous 512-col block st[:, b*512:(b+1)*512]
            stiles = []
            for k in range(KT):
                st = scaled_pool.tile([128, BC * 2 * S], BF16, tag="scaled")
                stv = st[:].rearrange("p (b h o) -> p b h o", b=BC, h=2)
                for h in range(2):
                    lbl = 2 * j + h
                    scale_op(
                        stv[:, :, h, :],
                        depT_v[:, :, k, :],
                        ut[:, k * L + lbl:k * L + lbl + 1],
                        wht[:, k * L + lbl:k * L + lbl + 1],
                    )
                stiles.append(st)
            for b in range(BC):
                osb = outp.tile([128, 4 * S], BF16, tag="osb")  # (l, ib, o)
                osb4 = osb[:].rearrange("i (l ib o) -> i l ib o", l=2, ib=2)
                for ib in range(2):
                    ps = mm_psum.tile([128, 2 * S], F32, tag="mm")
                    for k in range(KT):
                        hc = col(b, k) + ib * 128
                        nc.tensor.matmul(
                            ps[:],
                            headT[:, hc:hc + 128],
                            stiles[k][:, b * 2 * S:(b + 1) * 2 * S],
                            start=(k == 0),
                            stop=(k == KT - 1),
                        )
                    evac_add(
                        osb4[:, :, ib, :],
                        ps,
                        t2bc[:, b * ROWLEN + j * 2 * S:
                             b * ROWLEN + (j + 1) * 2 * S].rearrange(
                                 "p (l o) -> p l o", l=2),
                    )
                if split_last and b == BC - 1:
                    for c in range(4):
                        lh, ib = c // 2, c % 2
                        nc.sync.dma_start(
                            out_d[b, 2 * j + lh,
                                  ib * 128:(ib + 1) * 128, :],
                            osb[:, c * S:(c + 1) * S],
                        )
                else:
                    # one DMA per (b, pair): HBM dim (l,ib) has uniform stride
                    nc.sync.dma_start(
                        out_d[b, 2 * j:2 * j + 2, :, :].rearrange(
                            "l (ib i) o -> i (l ib) o", i=128),
                        osb[:].rearrange("i (lib o) -> i lib o", lib=4),
                    )

        # all transposes + t2 chains first so the 384-matmul main stream
        # then runs uninterrupted (keeps PE dense and HAM warm)
        load_transposed(dep_d, depT, 0)
        load_weightT(w_d[:, D:], wdt)
        t2_chain(0)
        load_weightT(u_d[:], ut)
        load_weightT(w_d[:, :D], wht)
        load_transposed(head_d, headT, 0)
        load_transposed(dep_d, depT, 1)
        t2_chain(1)
        load_transposed(head_d, headT, 1)
        for j in range(PAIRS):
            main_pair(j, split_last=(j == PAIRS - 1))

    nc.compile()
    return nc


def get_nc():
    if "nc" not in _NC_CACHE:
        _NC_CACHE["nc"] = _build_nc()
    return _NC_CACHE["nc"]


def make_in_maps(head, dep, u, w, bvec):
    head = np.ascontiguousarray(np.asarray(head, dtype=np.float32))
    dep = np.ascontiguousarray(np.asarray(dep, dtype=np.float32))
    u = np.ascontiguousarray(np.asarray(u, dtype=np.float32))
    w = np.ascontiguousarray(np.asarray(w, dtype=np.float32))
    bcol = np.ascontiguousarray(
        np.asarray(bvec, dtype=np.float32).reshape(L, 1)
    )
    return [
        {
            "head": head[c * BC:(c + 1) * BC],
            "dep": dep[c * BC:(c + 1) * BC],
            "u": u,
            "w": w,
            "b": bcol,
        }
        for c in range(NCORES)
    ]


def run(head, dep, label_U_diag, label_W, label_b, trace=False, **trace_kw):
    nc = get_nc()
    in_maps = make_in_maps(head, dep, label_U_diag, label_W, label_b)
    res = run_bass_kernel_spmd(
        nc, in_maps, core_ids=list(range(NCORES)), trace=trace, **trace_kw
    )
    out = np.concatenate(
        [np.asarray(res.results[c]["out"]).astype(np.float32)
         for c in range(NCORES)],
        axis=0,
    )
    return out, res


def kernel(**inputs):
    out, _ = run(
        inputs["head"],
        inputs["dep"],
        inputs["label_U_diag"],
        inputs["label_W"],
        inputs["label_b"],
    )
    return out


# revision 8
# speedup vs baseline: 1.3173x; 1.1345x over previous
"""BiaffineLabelAttention kernel for 8 TRN2 NeuronCores (Bass/Tile).

Reference computation (per full input):
    t1[b,l,i,o] = sum_d head[b,i,d] * U[l,d] * dep[b,o,d]
    t2_h[b,l,i] = sum_d W_h[l,d] * head[b,i,d]
    t2_d[b,l,o] = sum_d W_d[l,d] * dep[b,o,d]
    out = t1 + t2_h[...,None] + t2_d[...,None,:] + bias[l]

Sharding: data-parallel over batch (16 batches -> 2 per core x 8 cores).

Per-core algorithm (bf16 matmul datapath; tolerance is 2e-2 so bf16's
~5e-3 max rel err is fine and the PE runs 1 cycle/row vs fp32's 4):
    headT[d,(k,b,i)], depT[d,(k,b,o)] built via fp32 PE transposes,
        cast to bf16 during the PSUM->SBUF evacuation copy.
    st[j,k][d,(h,b,o)] = U[l]*depT + W_h[l]: one tensor_scalar per
        (j,k,label) with fully CONTIGUOUS src/dst APs (enables the DVE
        2x/4x bf16 packing modes); the MM rhs is then a 2-run strided
        view st[(h,:2),(o,:256)] at offset b*256.
    psum[i,(h2,o)] += headT[d,(k,b,ib)].T @ st  (bf16 matmuls, N=512)
    osb[(ib,l,o)] = psum + t2row  (DVE TT from PSUM, some via ACT
        copy + GpSimd add)
The W_h term telescopes through the matmul; t2row = t2_d + bias comes
from a small PE matmul, bounced through DRAM and broadcast to all 128
partitions by a single stride-0 DMA per batch (keeps GpSimd free).
Output is written bf16 and upcast to f32 on the host.
"""

import numpy as np
from contextlib import ExitStack

import concourse.bass as bass
from concourse import bacc, mybir, tile, masks
from concourse.bass_utils import run_bass_kernel_spmd

F32 = mybir.dt.float32
BF16 = mybir.dt.bfloat16

B, S, D, L = 16, 256, 768, 32
NCORES = 8
BC = B // NCORES          # batches per core
KT = D // 128             # contraction k-tiles
PAIRS = L // 2            # label pairs sharing one PSUM bank (N=512)
ROWLEN = L * S            # per-batch t2 row length (l,o) flattened

_NC_CACHE = {}


def _build_nc():
    nc = bacc.Bacc(
        "TRN2",
        target_bir_lowering=False,
        debug=False,
        enable_asserts=False,
        num_devices=NCORES,
    )
    head_d = nc.dram_tensor("head", [BC, S, D], F32, kind="ExternalInput")
    dep_d = nc.dram_tensor("dep", [BC, S, D], F32, kind="ExternalInput")
    u_d = nc.dram_tensor("u", [L, D], F32, kind="ExternalInput")
    w_d = nc.dram_tensor("w", [L, 2 * D], F32, kind="ExternalInput")
    b_d = nc.dram_tensor("b", [L, 1], F32, kind="ExternalInput")
    out_d = nc.dram_tensor("out", [BC, L, S, S], BF16, kind="ExternalOutput")
    t2_scratch = nc.dram_tensor("t2_scratch", [BC, L, S], BF16)

    with tile.TileContext(nc) as tc, ExitStack() as ctx:
        const = ctx.enter_context(tc.tile_pool(name="const", bufs=1))
        big = ctx.enter_context(tc.tile_pool(name="big", bufs=1))
        nat = ctx.enter_context(tc.tile_pool(name="nat", bufs=2))
        scaled_pool = ctx.enter_context(tc.tile_pool(name="scaled", bufs=14))
        outp = ctx.enter_context(tc.tile_pool(name="outp", bufs=6))
        evtmp = ctx.enter_context(tc.tile_pool(name="evtmp", bufs=3))
        tp_psum = ctx.enter_context(
            tc.tile_pool(name="tp_psum", bufs=2, space=bass.MemorySpace.PSUM)
        )
        t2_psum = ctx.enter_context(
            tc.tile_pool(name="t2_psum", bufs=1, space=bass.MemorySpace.PSUM)
        )
        mm_psum = ctx.enter_context(
            tc.tile_pool(name="mm_psum", bufs=5, space=bass.MemorySpace.PSUM)
        )

        ident = const.tile([128, 128], F32)
        masks.make_identity(nc, ident[:])

        headT = big.tile([128, KT * BC * S], BF16, tag="headT")  # [d,(k,b,i)]
        depT = big.tile([128, KT * BC * S], BF16, tag="depT")    # [d,(k,b,o)]
        ut = big.tile([128, KT * L], F32, tag="ut")    # col k*L+l = U[l,k-blk]
        wht = big.tile([128, KT * L], F32, tag="wht")
        wdt = big.tile([128, KT * L], BF16, tag="wdt")
        bias = const.tile([L, 1], F32, tag="bias")
        # t2bc[p, b*ROWLEN + l*S + o] = t2_d[b,l,o] + bias[l]; built in DRAM
        # then replicated to all partitions by a stride-0 broadcast DMA
        t2bc = big.tile([128, BC * ROWLEN], BF16, tag="t2bc")

        def col(b, k):
            return (k * BC + b) * S

        cp_idx = [0]

        def psum_copy(dst_ap, src_ap):
            # alternate DVE/ACT for PSUM->SBUF evacuation copies
            if cp_idx[0] % 2 == 0:
                nc.vector.tensor_copy(dst_ap, src_ap)
            else:
                nc.scalar.copy(dst_ap, src_ap)
            cp_idx[0] += 1

        nc.sync.dma_start(bias[:], b_d[:])

        def load_transposed(src_d, dst, b):
            # src [S, D] batch b  ->  dst[:, col(b,k) + i] = src[i, k*128+d]
            # 4+2 transposes share one psum bank; one strided copy out
            for ih in range(S // 128):
                natt = nat.tile([128, D], F32, tag="nat")
                half = D // 2
                nc.sync.dma_start(
                    natt[:, :half], src_d[b, ih * 128:(ih + 1) * 128, :half])
                nc.sync.dma_start(
                    natt[:, half:], src_d[b, ih * 128:(ih + 1) * 128, half:])
                for k0, nblk in ((0, 4), (4, 2)):
                    ps = tp_psum.tile([128, 4 * 128], F32, tag="tp")
                    for q in range(nblk):
                        k = k0 + q
                        nc.tensor.transpose(
                            ps[:, q * 128:(q + 1) * 128],
                            natt[:, k * 128:(k + 1) * 128], ident[:],
                        )
                    # dst columns for k0..k0+nblk at this ih: stride BC*S
                    out_ap = dst[:].rearrange(
                        "p (k b i) -> p k b i", b=BC, i=S
                    )[:, k0:k0 + nblk, b, ih * 128:(ih + 1) * 128]
                    psum_copy(
                        out_ap,
                        ps[:, :nblk * 128].rearrange(
                            "p (q i) -> p q i", i=128),
                    )

        def load_weightT(src_ap, dst):
            # src [L, D]  ->  dst[:, k*L + l] = src[l, k*128+d]
            natw = nat.tile([L, D], F32, tag="natw")
            nc.sync.dma_start(natw[:], src_ap)
            ps = tp_psum.tile([128, 4 * 128], F32, tag="tp")
            for k in range(KT):
                nc.tensor.transpose(
                    ps[:, k * L:(k + 1) * L],
                    natw[:, k * 128:(k + 1) * 128], ident[:L, :L]
                )
            psum_copy(dst[:], ps[:, :KT * L])

        def t2_chain(b):
            # t2row[b] = t2_d[b] + bias, replicated to all partitions
            ps = t2_psum.tile([L, S], F32, tag="t2")
            for k in range(KT):
                nc.tensor.matmul(
                    ps[:],
                    wdt[:, k * L:(k + 1) * L],
                    depT[:, col(b, k):col(b, k) + S],
                    start=(k == 0),
                    stop=(k == KT - 1),
                )
            t2sb = nat.tile([L, S], BF16, tag="t2sb")
            nc.vector.tensor_scalar_add(t2sb[:], ps[:], bias[:])
            nc.sync.dma_start(t2_scratch[b], t2sb[:])
            # one stride-0 DMA replicates the row to all 128 partitions
            nc.sync.dma_start(
                t2bc[:, b * ROWLEN:(b + 1) * ROWLEN],
                t2_scratch[b].rearrange("l o -> (l o)").partition_broadcast(
                    128),
            )

        op_idx = [0]

        def scale_op(dst_ap, src_ap, ucol, wcol):
            # st = U*depT + W_h for both batches; DVE/ACT/GpSimd split
            # (DVE gets 2x/4x bf16 packing so it takes the lion's share)
            r = op_idx[0] % 16
            if r in (5, 13, 15):
                nc.gpsimd.tensor_scalar(
                    dst_ap, src_ap, ucol, wcol,
                    mybir.AluOpType.mult, mybir.AluOpType.add,
                )
            elif r in (3, 7, 11):
                nc.scalar.activation(
                    dst_ap, src_ap,
                    mybir.ActivationFunctionType.Identity,
                    bias=wcol, scale=ucol,
                )
            else:
                nc.vector.tensor_scalar(
                    dst_ap, src_ap, ucol, wcol,
                    mybir.AluOpType.mult, mybir.AluOpType.add,
                )
            op_idx[0] += 1

        ev_idx = [0]

        def evac_add(osb_ap, ps, t2_ap):
            # osb = psum + t2row: mostly direct DVE TT (only DVE can
            # read+add from PSUM); every 4th via ACT copy + GpSimd add
            if ev_idx[0] % 4 == 2:
                tmp = evtmp.tile([128, 2 * S], BF16, tag="ev")
                nc.scalar.copy(tmp[:], ps[:])
                nc.gpsimd.tensor_tensor(
                    osb_ap, tmp[:], t2_ap, mybir.AluOpType.add,
                )
            else:
                nc.vector.tensor_tensor(
                    osb_ap, ps[:], t2_ap, mybir.AluOpType.add,
                )
            ev_idx[0] += 1

        def main_pair(j, split_last=False):
            # st[k][p, (h, b, o)]: one tensor_scalar per (k, label) with
            # flat contiguous APs covering both batches; MM rhs per batch
            # is the strided 2-run view (h:2, o:256) at offset b*256
            stiles = []
            for k in range(KT):
                st = scaled_pool.tile([128, 2 * BC * S], BF16, tag="scaled")
                for h in range(2):
                    lbl = 2 * j + h
                    scale_op(
                        st[:, h * BC * S:(h + 1) * BC * S],
                        depT[:, col(0, k):col(0, k) + BC * S],
                        ut[:, k * L + lbl:k * L + lbl + 1],
                        wht[:, k * L + lbl:k * L + lbl + 1],
                    )
                stiles.append(st)
            for b in range(BC):
                osb = outp.tile([128, 4 * S], BF16, tag="osb")  # (ib, l, o)
                for ib in range(2):
                    ps = mm_psum.tile([128, 2 * S], F32, tag="mm")
                    for k in range(KT):
                        hc = col(b, k) + ib * 128
                        nc.tensor.matmul(
                            ps[:],
                            headT[:, hc:hc + 128],
                            stiles[k][:].rearrange(
                                "p (h b o) -> p h b o", h=2, b=BC
                            )[:, :, b, :],
                            start=(k == 0),
                            stop=(k == KT - 1),
                        )
                    evac_add(
                        osb[:, ib * 2 * S:(ib + 1) * 2 * S],
                        ps,
                        t2bc[:, b * ROWLEN + j * 2 * S:
                             b * ROWLEN + (j + 1) * 2 * S],
                    )
                if split_last and b == BC - 1:
                    for c in range(4):
                        ib, lh = c // 2, c % 2
                        nc.sync.dma_start(
                            out_d[b, 2 * j + lh,
                                  ib * 128:(ib + 1) * 128, :],
                            osb[:, c * S:(c + 1) * S],
                        )
                else:
                    # one DMA per (b, pair, ib): HBM l-dim uniform stride
                    for ib in range(2):
                        nc.sync.dma_start(
                            out_d[b, 2 * j:2 * j + 2,
                                  ib * 128:(ib + 1) * 128, :].rearrange(
                                "l i o -> i l o"),
                            osb[:, ib * 2 * S:(ib + 1) * 2 * S].rearrange(
                                "i (l o) -> i l o", l=2),
                        )

        # all transposes + t2 chains first so the 384-matmul main stream
        # then runs uninterrupted (keeps PE dense and HAM warm)
        load_transposed(dep_d, depT, 0)
        load_weightT(w_d[:, D:], wdt)
        t2_chain(0)
        load_weightT(u_d[:], ut)
        load_weightT(w_d[:, :D], wht)
        load_transposed(head_d, headT, 0)
        load_transposed(dep_d, depT, 1)
        t2_chain(1)
        load_transposed(head_d, headT, 1)
        for j in range(PAIRS):
            main_pair(j, split_last=(j == PAIRS - 1))

    nc.compile()
    return nc


def get_nc():
    if "nc" not in _NC_CACHE:
        _NC_CACHE["nc"] = _build_nc()
    return _NC_CACHE["nc"]


def make_in_maps(head, dep, u, w, bvec):
    head = np.ascontiguousarray(np.asarray(head, dtype=np.float32))
    dep = np.ascontiguousarray(np.asarray(dep, dtype=np.float32))
    u = np.ascontiguousarray(np.asarray(u, dtype=np.float32))
    w = np.ascontiguousarray(np.asarray(w, dtype=np.float32))
    bcol = np.ascontiguousarray(
        np.asarray(bvec, dtype=np.float32).reshape(L, 1)
    )
    return [
        {
            "head": head[c * BC:(c + 1) * BC],
            "dep": dep[c * BC:(c + 1) * BC],
            "u": u,
            "w": w,
            "b": bcol,
        }
        for c in range(NCORES)
    ]


def run(head, dep, label_U_diag, label_W, label_b, trace=False, **trace_kw):
    nc = get_nc()
    in_maps = make_in_maps(head, dep, label_U_diag, label_W, label_b)
    res = run_bass_kernel_spmd(
        nc, in_maps, core_ids=list(range(NCORES)), trace=trace, **trace_kw
    )
    out = np.concatenate(
        [np.asarray(res.results[c]["out"]).astype(np.float32)
         for c in range(NCORES)],
        axis=0,
    )
    return out, res


def kernel(**inputs):
    out, _ = run(
        inputs["head"],
        inputs["dep"],
        inputs["label_U_diag"],
        inputs["label_W"],
        inputs["label_b"],
    )
    return out
